# revision 2
# baseline (speedup 1.0000x reference)
"""YOLO-style detection decode on 8 Trainium2 NeuronCores.

Data-parallel over batch: core i handles images [4i, 4i+4).  Per (image,
scale): the [255, HW] channel-major feature map is PE-transposed in 64-row
slabs into a cells-on-partitions PSUM layout [128 cells, chunk, 255] where
channel c sits at column c (anchor a's fields at a*85..a*85+85).  DVE then
computes the per-cell class argmax batched over all 3 anchors
(reduce_max -> is_ge mask -> mask*revidx -> reduce_max; exact,
first-occurrence tie-safe) while ACT does exp(dw + ln(anchor)) and conf
copies.  Outputs are assembled per (image, scale) as [128, nch, 3, 6]
tiles and stored tile-major with one contiguous DMA each; the host gather
permutes to the reference row order.
"""

import sys
from contextlib import ExitStack

import numpy as np

if "/opt/trn_rl_repo" not in sys.path:
    sys.path.insert(0, "/opt/trn_rl_repo")

NCORES = 8
B = 32
BLOC = B // NCORES  # images per core
NF = 85  # fields per anchor: conf, dx, dy, dw, dh, 80 class logits
NCLS = 80
NANCH = 3
CCOL = 256  # padded chunk stride in PSUM columns (255 fields + 1 pad)
PGRP = 8  # chunks per PSUM group tile (8 * 256 * 4B = 4 banks)

# (name, H, W, HW, step, thresh, nchunks)
SCALES = [
    ("x13", 13, 13, 169, 32.0, 0.5, 2),
    ("x26", 26, 26, 676, 16.0, 0.5, 6),
    ("x52", 52, 52, 2704, 8.0, 0.9, 22),
]
ROWS_PER_B = sum(hw * NANCH for _, _, _, hw, _, _, _ in SCALES)  # 10647
# Device output is tile-major: per (b, scale) a [128, nch*18] block, flattened.
TILE_BLOCK = {name: 128 * nch * 18 for name, _, _, _, _, _, nch in SCALES}
OUT_FLAT = BLOC * sum(TILE_BLOCK.values())  # 276480

_PROG_CACHE = {}


def _out_offset(b, s):
    per_b = sum(TILE_BLOCK.values())
    ofs = b * per_b
    for j in range(s):
        ofs += TILE_BLOCK[SCALES[j][0]]
    return ofs


def _groups(nch):
    out = []
    g0 = 0
    while g0 < nch:
        out.append((g0, min(PGRP, nch - g0)))
        g0 += PGRP
    return out


def _build_program():
    import concourse.bacc as bacc
    import concourse.mybir as mybir
    from concourse.tile import TileContext

    f32 = mybir.dt.float32
    AL = mybir.AluOpType
    AF = mybir.ActivationFunctionType
    AX = mybir.AxisListType

    nc = bacc.Bacc("TRN2", target_bir_lowering=False, debug=False)

    xin = {}
    for name, _, _, hw, _, _, _ in SCALES:
        xin[name] = nc.dram_tensor(
            name, [BLOC, 255, hw], f32, kind="ExternalInput"
        ).ap()
    c_ident = nc.dram_tensor("c_ident", [128, 128], f32, kind="ExternalInput").ap()
    c_anch = nc.dram_tensor("c_anch", [128, 18], f32, kind="ExternalInput").ap()
    c_revidx = nc.dram_tensor("c_revidx", [128, NCLS], f32, kind="ExternalInput").ap()
    c_gxy = {}
    for name, _, _, _, _, _, nch in SCALES:
        c_gxy[name] = nc.dram_tensor(
            f"c_gxy_{name}", [128, nch, 2], f32, kind="ExternalInput"
        ).ap()
    out = nc.dram_tensor("out", [OUT_FLAT], f32, kind="ExternalOutput").ap()

    with TileContext(nc) as tc, ExitStack() as ctx:
        const = ctx.enter_context(tc.tile_pool(name="const", bufs=1))
        ident_t = const.tile([128, 128], f32)
        nc.sync.dma_start(ident_t[:], c_ident[:])
        anch_t = const.tile([128, 18], f32)
        nc.sync.dma_start(anch_t[:], c_anch[:])
        lnanch_t = const.tile([128, 18], f32)
        nc.scalar.activation(lnanch_t[:], anch_t[:], AF.Ln)
        revidx_t = const.tile([128, NCLS], f32)
        nc.sync.dma_start(revidx_t[:], c_revidx[:])
        gxy_t = {}
        for name, _, _, _, _, _, nch in SCALES:
            t = const.tile([128, nch * 2], f32, tag=f"gxy_{name}")
            nc.sync.dma_start(
                t[:].rearrange("p (g j) -> p g j", j=2), c_gxy[name][:]
            )
            gxy_t[name] = t

        in_pool = ctx.enter_context(tc.tile_pool(name="inp", bufs=8))
        ps_pool = ctx.enter_context(tc.tile_pool(name="ps", bufs=2, space="PSUM"))
        wk = ctx.enter_context(tc.tile_pool(name="wk", bufs=2))
        op = ctx.enter_context(tc.tile_pool(name="op", bufs=2))

        for b in range(BLOC):
            for s, (name, Hh, Ww, HW, step, thresh, nch) in enumerate(SCALES):
                x = xin[name]

                O = op.tile([128, nch * 18], f32, tag=f"O{s}")
                O4 = O[:].rearrange("p (g a f) -> p g a f", a=3, f=6)
                O3 = O[:].rearrange("p (ga f) -> p ga f", f=6)
                M_t = wk.tile([128, nch * 3], f32, tag="M")
                m_t = wk.tile([128, nch * 3], f32, tag="m")
                mv = m_t[:].rearrange("p (g a) -> p g a", a=3)
                r_t = wk.tile([128, nch * 3], f32, tag="r")
                wh_t = wk.tile([128, nch * 6], f32, tag="wh")
                whv = wh_t[:].rearrange("p (g a j) -> p g a j", a=3, j=2)
                u_t = wk.tile([128, nch * 6], f32, tag="u")
                uv = u_t[:].rearrange("p (g a j) -> p g a j", a=3, j=2)
                gxyv = gxy_t[name][:].rearrange("p (g j) -> p g j", j=2)

                for g0, gch in _groups(nch):
                    # per-group input tiles: released as soon as this group's
                    # transposes have read them, so loads stream ahead
                    gw = min(HW, (g0 + gch) * 128) - g0 * 128
                    T0 = in_pool.tile([128, PGRP * 128], f32, tag="T0")
                    T1 = in_pool.tile([127, PGRP * 128], f32, tag="T1")
                    nc.gpsimd.dma_start(
                        T0[:, 0:gw], x[b, 0:128, g0 * 128 : g0 * 128 + gw]
                    )
                    nc.gpsimd.dma_start(
                        T1[:, 0:gw], x[b, 128:255, g0 * 128 : g0 * 128 + gw]
                    )
                    P = ps_pool.tile([128, PGRP * CCOL], f32, tag="P")
                    for c in range(gch):
                        gc = g0 + c
                        cells = min(128, HW - gc * 128)
                        col = c * 128
                        fo = c * CCOL
                        if cells < 128:
                            # tail chunk: pre-zero so pad partitions are
                            # defined (transposes overwrite the valid rows;
                            # WAW dep orders the memset first)
                            nc.vector.memset(P[:, fo : fo + 255], 0.0)
                        # channels 0..255 -> psum cols fo+0..fo+255.  Both
                        # transposes use tile_position (0,0): mixing base-0
                        # and base-64 matmul positions on one PSUM bank is a
                        # fatal HW error.
                        nc.tensor.transpose(
                            P[0:cells, fo : fo + 128],
                            T0[:, col : col + cells],
                            ident_t[:, :],
                        )
                        nc.tensor.transpose(
                            P[0:cells, fo + 128 : fo + 255],
                            T1[:, col : col + cells],
                            ident_t[0:127, 0:127],
                        )
                    # [128, gch, 3, 85] view: anchor a's fields at col a*85+f
                    P4 = (
                        P[:, :]
                        .rearrange("p (g f) -> p g f", f=CCOL)[:, 0:gch, 0:255]
                        .rearrange("p g (a f) -> p g a f", f=NF)
                    )
                    logits = P4[:, :, :, 5:]
                    gs = slice(g0, g0 + gch)
                    # reduce allows 4D input (XYZW); out is the [g, a] slice
                    m_g = m_t[:, g0 * 3 : (g0 + gch) * 3].rearrange(
                        "p (g a) -> p g a", a=3
                    )
                    nc.vector.tensor_reduce(
                        out=m_g, in_=logits, axis=AX.X, op=AL.max
                    )
                    mask_t = wk.tile([128, PGRP * 3 * NCLS], f32, tag="mask")
                    mask4 = mask_t[:, 0 : gch * 3 * NCLS].rearrange(
                        "p (g a k) -> p g a k", a=3, k=NCLS
                    )
                    mask3 = mask_t[:, 0 : gch * 3 * NCLS].rearrange(
                        "p (ga k) -> p ga k", k=NCLS
                    )
                    # mask = (logits + 0) >= m  (1.0/0.0); stt APs must be <=3D
                    for a in range(NANCH):
                        nc.vector.scalar_tensor_tensor(
                            out=mask4[:, :, a, :],
                            in0=P4[:, :, a, 5:],
                            scalar=0.0,
                            in1=mv[:, gs, a]
                            .unsqueeze(2)
                            .broadcast_to([128, gch, NCLS]),
                            op0=AL.add,
                            op1=AL.is_ge,
                        )
                    # v = mask * (80 - j); reduce_max -> 80 - first argmax
                    nc.vector.tensor_tensor(
                        out=mask3,
                        in0=mask3,
                        in1=revidx_t[:]
                        .unsqueeze(1)
                        .broadcast_to([128, gch * 3, NCLS]),
                        op=AL.mult,
                    )
                    nc.vector.tensor_reduce(
                        out=r_t[:, g0 * 3 : (g0 + gch) * 3],
                        in_=mask3,
                        axis=AX.X,
                        op=AL.max,
                    )
                    # box math, per anchor (stt/ACT need <=3D APs)
                    for a in range(NANCH):
                        colw = s * 6 + a * 2
                        nc.scalar.activation(
                            out=whv[:, gs, a, 0:1],
                            in_=P4[:, :, a, 3:4],
                            func=AF.Exp,
                            bias=lnanch_t[:, colw : colw + 1],
                        )
                        nc.scalar.activation(
                            out=whv[:, gs, a, 1:2],
                            in_=P4[:, :, a, 4:5],
                            func=AF.Exp,
                            bias=lnanch_t[:, colw + 1 : colw + 2],
                        )
                        # u = dxy*step + g*step
                        nc.vector.scalar_tensor_tensor(
                            out=uv[:, gs, a, :],
                            in0=P4[:, :, a, 1:3],
                            scalar=step,
                            in1=gxyv[:, gs],
                            op0=AL.mult,
                            op1=AL.add,
                        )
                        # xy1 = u - 0.5*wh
                        nc.vector.scalar_tensor_tensor(
                            out=O4[:, gs, a, 1:3],
                            in0=whv[:, gs, a, :],
                            scalar=-0.5,
                            in1=uv[:, gs, a, :],
                            op0=AL.mult,
                            op1=AL.add,
                        )
                        # xy2 = xy1 + wh
                        nc.vector.tensor_tensor(
                            out=O4[:, gs, a, 3:5],
                            in0=O4[:, gs, a, 1:3],
                            in1=whv[:, gs, a, :],
                            op=AL.add,
                        )
                        # conf copy
                        nc.scalar.activation(
                            out=O4[:, gs, a, 0:1],
                            in_=P4[:, :, a, 0:1],
                            func=AF.Copy,
                        )
                # cls = 80 - r
                nc.vector.tensor_scalar(
                    out=O3[:, :, 5:6],
                    in0=r_t[:].unsqueeze(2),
                    scalar1=-1.0,
                    scalar2=80.0,
                    op0=AL.mult,
                    op1=AL.add,
                )
                # cell mask: conf > thresh
                nc.vector.tensor_scalar(
                    out=M_t[:, :],
                    in0=O3[:, :, 0],
                    scalar1=thresh,
                    scalar2=None,
                    op0=AL.is_gt,
                )
                # zero masked cells
                nc.vector.tensor_tensor(
                    out=O3,
                    in0=O3,
                    in1=M_t[:].unsqueeze(2).broadcast_to([128, nch * 3, 6]),
                    op=AL.mult,
                )
                ofs = _out_offset(b, s)
                w = nch * 18
                dst = out[ofs : ofs + 128 * w].rearrange("(p w) -> p w", w=w)
                nc.sync.dma_start(dst, O[:, :])
    nc.compile()
    return nc


def _host_constants(anchors):
    consts = {
        "c_ident": np.eye(128, dtype=np.float32),
        "c_anch": np.ascontiguousarray(
            np.broadcast_to(
                np.asarray(anchors, dtype=np.float32).reshape(1, 18), (128, 18)
            )
        ),
        "c_revidx": np.ascontiguousarray(
            np.broadcast_to(
                (80.0 - np.arange(NCLS, dtype=np.float32)).reshape(1, NCLS),
                (128, NCLS),
            )
        ),
    }
    for name, Hh, Ww, HW, step, thresh, nch in SCALES:
        g = np.zeros((128, nch, 2), dtype=np.float32)
        for c in range(nch):
            for p in range(128):
                hw = c * 128 + p
                if hw < HW:
                    g[p, c, 0] = (hw % Ww) * step
                    g[p, c, 1] = (hw // Ww) * step
        consts[f"c_gxy_{name}"] = g
    return consts


def kernel(output13, output26, output52, anchors):
    from concourse.bass_utils import run_bass_kernel_spmd

    if "nc" not in _PROG_CACHE:
        _PROG_CACHE["nc"] = _build_program()
    nc = _PROG_CACHE["nc"]

    consts = _host_constants(np.asarray(anchors, dtype=np.float32))
    xs = {
        "x13": np.asarray(output13, dtype=np.float32).reshape(B, 255, 169),
        "x26": np.asarray(output26, dtype=np.float32).reshape(B, 255, 676),
        "x52": np.asarray(output52, dtype=np.float32).reshape(B, 255, 2704),
    }
    in_maps = []
    for i in range(NCORES):
        m = dict(consts)
        for k, v in xs.items():
            m[k] = np.ascontiguousarray(v[i * BLOC : (i + 1) * BLOC])
        in_maps.append(m)

    res = run_bass_kernel_spmd(nc, in_maps, core_ids=list(range(NCORES)))
    _PROG_CACHE["last_res"] = res

    full = np.zeros((B * ROWS_PER_B, 6), np.float32)
    scale_full_base = [0, B * 169 * 3, B * 169 * 3 + B * 676 * 3]
    for i in range(NCORES):
        o = np.asarray(res.results[i]["out"]).reshape(-1)
        for b in range(BLOC):
            for s, (name, Hh, Ww, HW, step, thresh, nch) in enumerate(SCALES):
                ofs = _out_offset(b, s)
                seg = o[ofs : ofs + 128 * nch * 18].reshape(128, nch, 3, 6)
                rows = seg.transpose(1, 0, 2, 3).reshape(nch * 128 * 3, 6)
                gb = scale_full_base[s] + (i * BLOC + b) * HW * 3
                full[gb : gb + HW * 3] = rows[: HW * 3]
    return full

